# revision 3
# baseline (speedup 1.0000x reference)
"""Trainium2 Bass kernel v6 for nn_LocalAttn: grouped local attention (3x3).

Sharding: 8 cores = batch(2) x H-strips(4), 34 input rows (1-row halo),
32 output rows, W padded to 130. Channel-major (channels on partitions).

v6 vs v5: softmax + apply are pipelined in TWO column-halves so the
serial softmax chain of half B (logits->exp->denom->ln->1/den->bcast->
a72) runs on Scalar/PE underneath half A's apply muls on DVE, and the
apply starts ~15us earlier.
"""

import numpy as np
import ml_dtypes

import concourse.bass as bass
import concourse.bacc as bacc
import concourse.mybir as mybir
from concourse import tile
from concourse.bass_utils import run_bass_kernel_spmd

F32 = mybir.dt.float32
BF16 = mybir.dt.bfloat16
AF = mybir.ActivationFunctionType
ALU = mybir.AluOpType

EPS = 1e-5
G = 8
B = 2
C = 256
H = W = 128
HS = 32
HI = 34
WP = 130
NIN = HI * WP          # 4420
NOUT = HS * WP         # 4160
NPAD = NIN + 2         # 4422, v/nbr base-1
CT = 442               # conv pixel tile (10 over 4420)
NCT = NIN // CT

# column halves of the output window
HALVES = [(0, 2048), (2048, 2112)]
# psum chunking within a half (chunks of <=1024, bank-pair aligned)
def _chunks(n0, nn):
    out = []
    c = 0
    while c < nn:
        s = min(1024, nn - c)
        out.append((n0 + c, s))
        c += s
    return out


def _koff(k):
    dy, dx = k // 3 - 1, k % 3 - 1
    return 1 + WP + dy * WP + dx


_NC_CACHE = {}


def _build_nc():
    nc = bacc.Bacc("TRN2", target_bir_lowering=False, debug=False, num_devices=8)

    x_d = nc.dram_tensor("x", [2, 128, NIN], F32, kind="ExternalInput")
    w1_d = nc.dram_tensor("w1bd", [2, 128, 32], BF16, kind="ExternalInput")
    s1_d = nc.dram_tensor("s1", [64, 1], F32, kind="ExternalInput")
    c1_d = nc.dram_tensor("c1", [64, 1], F32, kind="ExternalInput")
    w2_d = nc.dram_tensor("w2c", [64, 80], BF16, kind="ExternalInput")
    s2_d = nc.dram_tensor("s2c", [80, 1], F32, kind="ExternalInput")
    c2_d = nc.dram_tensor("c2c", [80, 1], F32, kind="ExternalInput")
    wv_d = nc.dram_tensor("wvh", [4, 128, 128], BF16, kind="ExternalInput")
    ds_d = nc.dram_tensor("dsum", [72, 8], BF16, kind="ExternalInput")
    bc_d = nc.dram_tensor("bc72", [8, 72], BF16, kind="ExternalInput")
    sel_d = nc.dram_tensor("selk", [9, 72, 128], BF16, kind="ExternalInput")
    hm_d = nc.dram_tensor("hmask", [8, 2 * WP], F32, kind="ExternalInput")
    out_d = nc.dram_tensor("out", [2, 128, NOUT], BF16, kind="ExternalOutput")

    import os
    with tile.TileContext(nc, linearize=bool(os.environ.get("K_LINEARIZE"))) as tc:
        with (
            tc.tile_pool(name="const", bufs=1) as cp,
            tc.tile_pool(name="persist", bufs=1) as pp,
        ):
            # ---- hot constants first ----
            w1t = []
            for q in range(2):
                t_ = cp.tile([128, 32], BF16, tag=f"w1_{q}", name=f"w1_{q}")
                nc.sync.dma_start(t_[:], w1_d[q])
                w1t.append(t_)
            s1t = cp.tile([64, 1], F32, tag="s1", name="s1")
            nc.sync.dma_start(s1t[:], s1_d[:])
            c1t = cp.tile([64, 1], F32, tag="c1", name="c1")
            nc.sync.dma_start(c1t[:], c1_d[:])
            # w2/s2/c2/hm arrive during the first conv tiles
            w2t = cp.tile([64, 80], BF16, tag="w2c", name="w2c")
            s2t = cp.tile([80, 1], F32, tag="s2c", name="s2c")
            c2t = cp.tile([80, 1], F32, tag="c2c", name="c2c")
            hmt = cp.tile([8, 2 * WP], F32, tag="hm", name="hm")
            nc.gpsimd.dma_start(w2t[:], w2_d[:])
            nc.gpsimd.dma_start(s2t[:], s2_d[:])
            nc.gpsimd.dma_start(c2t[:], c2_d[:])
            nc.gpsimd.dma_start(hmt[:], hm_d[:])
            # ---- cold constants on the gpsimd queue ----
            wvbt, selt = [], []
            for j in range(4):
                t_ = cp.tile([128, 128], BF16, tag=f"wv_{j}", name=f"wv_{j}")
                nc.gpsimd.dma_start(t_[:], wv_d[j])
                wvbt.append(t_)
            for k in range(9):
                t_ = cp.tile([72, 128], BF16, tag=f"sel_{k}", name=f"sel_{k}")
                nc.gpsimd.dma_start(t_[:], sel_d[k])
                selt.append(t_)
            dst = cp.tile([72, 8], BF16, tag="dsum", name="dsum")
            nc.gpsimd.dma_start(dst[:], ds_d[:])
            bct = cp.tile([8, 72], BF16, tag="bc72", name="bc72")
            nc.gpsimd.dma_start(bct[:], bc_d[:])

            # ---- PE warm-up: keep HAM busy during the input-DMA ramp ----
            with tc.tile_pool(name="warm", bufs=1, space="PSUM") as wps:
                wp_ = wps.tile([32, 32], F32)
                for _ in range(60):
                    nc.tensor.matmul(wp_[:], w1t[0][:, 0:32], w1t[0][:, 0:32])
            # ---- persistent across phases ----
            a72 = pp.tile([72, NOUT], BF16, tag="a72", name="a72")
            vh = [pp.tile([128, NPAD], BF16, tag=f"v_{h}", name=f"v_{h}")
                  for h in range(2)]

            with tc.tile_pool(name="mid", bufs=1) as mp:
                mn80 = mp.tile([80, NIN], F32, tag="mn80", name="mn80")
                nbrp = mp.tile([8, NPAD], F32, tag="nbrp", name="nbrp")
                nb72 = mp.tile([72, NOUT], F32, tag="nb72", name="nb72")
                e72 = mp.tile([72, NOUT], BF16, tag="e72", name="e72")
                lnd = mp.tile([8, 2112], F32, tag="lnd", name="lnd")
                rb = mp.tile([8, NOUT], BF16, tag="rb", name="rb")
                rb72 = mp.tile([72, NOUT], BF16, tag="rb72", name="rb72")

                with tc.tile_pool(name="xin", bufs=1) as xp:
                    xq, xb = [], []
                    for q in range(2):
                        xt = xp.tile([128, NIN], F32, tag=f"x_{q}", name=f"x_{q}")
                        xq.append(xt)
                        xbt = xp.tile([128, NIN], BF16, tag=f"xb_{q}",
                                      name=f"xb_{q}")
                        xb.append(xbt)
                    xqueues = [nc.sync, nc.scalar]
                    for c0 in range(0, NIN, 2 * CT):
                        cn = min(2 * CT, NIN - c0)
                        for q in range(2):
                            xqueues[q].dma_start(xq[q][:, c0: c0 + cn],
                                                 x_d[q, :, c0: c0 + cn])
                    t_sb = xp.tile([64, NIN], BF16, tag="t", name="t")

                    with tc.tile_pool(name="pv", bufs=2, space="PSUM") as pvp:
                        with (
                            tc.tile_pool(name="pc64", bufs=2, space="PSUM") as pc64,
                            tc.tile_pool(name="pc80", bufs=2, space="PSUM") as pc80,
                        ):
                            for it in range(NCT):
                                sl = slice(it * CT, (it + 1) * CT)
                                nc.vector.tensor_copy(xb[0][:, sl],
                                                      xq[0][:, sl])
                                nc.vector.tensor_copy(xb[1][:, sl],
                                                      xq[1][:, sl])
                                pt = pc64.tile([64, CT], F32)
                                nc.tensor.matmul(pt[0:32, :], w1t[0][:],
                                                 xb[0][:, sl], start=True,
                                                 stop=True, tile_position=(0, 0))
                                nc.tensor.matmul(pt[32:64, :], w1t[1][:],
                                                 xb[1][:, sl], start=True,
                                                 stop=True, tile_position=(0, 32))
                                nc.scalar.activation(t_sb[:, sl], pt[:], AF.Tanh,
                                                     bias=c1t[:, 0:1],
                                                     scale=s1t[:, 0:1])
                                p8 = pc80.tile([80, CT], F32)
                                nc.tensor.matmul(p8[:], w2t[:], t_sb[:, sl])
                                nc.scalar.activation(mn80[:, sl], p8[:],
                                                     AF.Identity,
                                                     bias=c2t[:, 0:1],
                                                     scale=s2t[:, 0:1])
                                nc.sync.dma_start(
                                    nbrp[:, 1 + it * CT: 1 + (it + 1) * CT],
                                    mn80[72:80, sl])

                        # neighbor fixups
                        nc.gpsimd.memset(nbrp[:, 0:1], 0.0)
                        nc.gpsimd.memset(nbrp[:, NPAD - 1: NPAD], 0.0)
                        nc.gpsimd.memset(nbrp[0:8, 1: 1 + HI * WP: WP], 0.0)
                        nc.gpsimd.memset(nbrp[0:8, 1 + WP - 1: 1 + HI * WP: WP],
                                         0.0)
                        nc.vector.tensor_mul(nbrp[0:8, 1: 1 + WP],
                                             nbrp[0:8, 1: 1 + WP], hmt[:, 0:WP])
                        nc.vector.tensor_mul(nbrp[0:8, 1 + 33 * WP: 1 + 34 * WP],
                                             nbrp[0:8, 1 + 33 * WP: 1 + 34 * WP],
                                             hmt[:, WP: 2 * WP])

                        def value_conv(h):
                            for it in range(NCT):
                                sl = slice(it * CT, (it + 1) * CT)
                                pv = pvp.tile([128, CT], F32)
                                nc.tensor.matmul(pv[:], wvbt[2 * h + 0][:],
                                                 xb[0][:, sl],
                                                 start=True, stop=False)
                                nc.tensor.matmul(pv[:], wvbt[2 * h + 1][:],
                                                 xb[1][:, sl],
                                                 start=False, stop=True)
                                dsl = slice(1 + it * CT, 1 + (it + 1) * CT)
                                nc.vector.tensor_copy(vh[h][:, dsl], pv[:])

                        def softmax_half(ha, pden, pb72):
                            n0, nn = HALVES[ha]
                            qs = ([nc.sync, nc.gpsimd, nc.scalar]
                                  if ha == 0 else [nc.sync, nc.gpsimd])
                            for k in range(9):
                                off = _koff(k) + n0
                                qs[k % len(qs)].dma_start(
                                    nb72[8 * k: 8 * k + 8, n0: n0 + nn],
                                    nbrp[0:8, off: off + nn])
                            sl = slice(n0, n0 + nn)
                            nc.vector.tensor_add(
                                nb72[:, sl], mn80[0:72, WP + n0: WP + n0 + nn],
                                nb72[:, sl])
                            nc.scalar.activation(e72[:, sl], nb72[:, sl], AF.Exp)
                            for c0, cn in _chunks(n0, nn):
                                for m0 in range(0, cn, 512):
                                    mn_ = min(512, cn - m0)
                                    s2 = slice(c0 + m0, c0 + m0 + mn_)
                                    sloc = slice(c0 + m0 - n0,
                                                 c0 + m0 - n0 + mn_)
                                    pd = pden.tile([8, 512], F32)
                                    nc.tensor.matmul(pd[:, 0:mn_], dst[:],
                                                     e72[:, s2])
                                    nc.scalar.activation(lnd[:, sloc],
                                                         pd[:, 0:mn_], AF.Ln)
                            nc.scalar.activation(rb[:, sl], lnd[:, 0:nn],
                                                 AF.Exp, scale=-1.0)
                            if ha == 0:
                                # latency-critical half: multiply straight
                                # from PSUM, skipping the scalar eviction hop
                                for c0, cn in _chunks(n0, nn):
                                    for m0 in range(0, cn, 512):
                                        mn_ = min(512, cn - m0)
                                        s2 = slice(c0 + m0, c0 + m0 + mn_)
                                        pb = pb72.tile([72, 512], F32)
                                        nc.tensor.matmul(pb[:, 0:mn_], bct[:],
                                                         rb[:, s2])
                                        nc.vector.tensor_mul(a72[:, s2],
                                                             e72[:, s2],
                                                             pb[:, 0:mn_])
                            else:
                                for c0, cn in _chunks(n0, nn):
                                    for m0 in range(0, cn, 512):
                                        mn_ = min(512, cn - m0)
                                        s2 = slice(c0 + m0, c0 + m0 + mn_)
                                        pb = pb72.tile([72, 512], F32)
                                        nc.tensor.matmul(pb[:, 0:mn_], bct[:],
                                                         rb[:, s2])
                                        nc.scalar.copy(rb72[:, s2],
                                                       pb[:, 0:mn_])
                                nc.vector.tensor_mul(a72[:, sl], e72[:, sl],
                                                     rb72[:, sl])

                        value_conv(0)
                        with (
                            tc.tile_pool(name="pdA", bufs=2, space="PSUM") as pdA,
                            tc.tile_pool(name="pbA", bufs=2, space="PSUM") as pbA,
                        ):
                            softmax_half(0, pdA, pbA)
                            value_conv(1)

                # xin + pv closed; apply scratch + softmax B interleaved
                with tc.tile_pool(name="apl", bufs=1) as ap:
                    vo = [ap.tile([128, NPAD - 1], BF16, tag=f"vo_{h}",
                                  name=f"vo_{h}") for h in range(2)]
                    HMAX = max(nn for _, nn in HALVES)
                    ek = [ap.tile([128, HMAX], BF16, tag=f"ek_{j}",
                                  name=f"ek_{j}") for j in range(6)]
                    pr = [ap.tile([128, HMAX], BF16, tag=f"pr_{j}",
                                  name=f"pr_{j}") for j in range(2)]
                    acc = [ap.tile([128, NOUT], BF16, tag=f"acc_{h}",
                                   name=f"acc_{h}") for h in range(2)]

                    for h in range(2):
                        nc.gpsimd.memset(vh[h][:, 0:1], 0.0)
                        nc.gpsimd.memset(vh[h][:, NPAD - 1: NPAD], 0.0)
                    for h in range(2):
                        nc.vector.tensor_copy(vo[h][:], vh[h][:, 1:NPAD])

                    def apply_grp(ha, grp, pap):
                        n0, nn = HALVES[ha]
                        hs = slice(n0, n0 + nn)
                        ks = [3 * grp, 3 * grp + 1, 3 * grp + 2]
                        eks = [ek[(9 * ha + 3 * grp + j) % 6] for j in range(3)]
                        for j, k in enumerate(ks):
                            for c0, cn in _chunks(n0, nn):
                                pa = pap.tile([128, 1024], F32)
                                for m0 in range(0, cn, 512):
                                    mn_ = min(512, cn - m0)
                                    nc.tensor.matmul(
                                        pa[:, m0: m0 + mn_], selt[k][:],
                                        a72[:, c0 + m0: c0 + m0 + mn_])
                                nc.scalar.copy(
                                    eks[j][:, c0 - n0: c0 - n0 + cn],
                                    pa[:, 0:cn])
                        def vsl(k, h):
                            off = _koff(k)
                            if off % 2 == 0:
                                return vh[h][:, off + n0: off + n0 + nn]
                            return vo[h][:, off - 1 + n0: off - 1 + n0 + nn]
                        for h in range(2):
                            nc.vector.tensor_mul(pr[0][:, 0:nn],
                                                 eks[0][:, 0:nn], vsl(ks[0], h))
                            nc.vector.tensor_mul(pr[1][:, 0:nn],
                                                 eks[1][:, 0:nn], vsl(ks[1], h))
                            nc.vector.tensor_add(pr[0][:, 0:nn], pr[0][:, 0:nn],
                                                 pr[1][:, 0:nn])
                            nc.vector.tensor_mul(pr[1][:, 0:nn],
                                                 eks[2][:, 0:nn], vsl(ks[2], h))
                            if grp == 0:
                                nc.vector.tensor_add(acc[h][:, hs],
                                                     pr[0][:, 0:nn],
                                                     pr[1][:, 0:nn])
                            elif grp == 1:
                                nc.vector.tensor_add(pr[0][:, 0:nn],
                                                     pr[0][:, 0:nn],
                                                     pr[1][:, 0:nn])
                                nc.vector.tensor_add(acc[h][:, hs],
                                                     acc[h][:, hs],
                                                     pr[0][:, 0:nn])
                            else:
                                # final group: chunk the last adds so the
                                # out-DMA can start on the first chunk early
                                nc.vector.tensor_add(pr[0][:, 0:nn],
                                                     pr[0][:, 0:nn],
                                                     pr[1][:, 0:nn])
                                oq = [nc.sync, nc.scalar]
                                half = (nn // 2) & ~1
                                for ci, (f0, fn) in enumerate(
                                        [(0, half), (half, nn - half)]):
                                    nc.vector.tensor_add(
                                        acc[h][:, n0 + f0: n0 + f0 + fn],
                                        acc[h][:, n0 + f0: n0 + f0 + fn],
                                        pr[0][:, f0: f0 + fn])
                                    oq[(h + ci) % 2].dma_start(
                                        out_d[h, :, n0 + f0: n0 + f0 + fn],
                                        acc[h][:, n0 + f0: n0 + f0 + fn])

                    with (
                        tc.tile_pool(name="pa", bufs=3, space="PSUM") as pap,
                        tc.tile_pool(name="pdB", bufs=1, space="PSUM") as pdB,
                        tc.tile_pool(name="pbB", bufs=1, space="PSUM") as pbB,
                    ):
                        apply_grp(0, 0, pap)
                        softmax_half(1, pdB, pbB)
                        apply_grp(0, 1, pap)
                        apply_grp(0, 2, pap)
                        apply_grp(1, 0, pap)
                        apply_grp(1, 1, pap)
                        apply_grp(1, 2, pap)

    nc.compile()
    return nc


def _host_prep(x, w1, b1, g1, be1, m1, v1, w2, b2, g2, be2, m2, v2, wv):
    f32 = np.float32
    bf16 = ml_dtypes.bfloat16

    inv1 = (g1 / np.sqrt(v1 + EPS)).astype(f32)
    s1 = inv1
    c1 = (b1 * inv1 + be1 - m1 * inv1).astype(f32)
    inv2 = (g2 / np.sqrt(v2 + EPS)).astype(f32)
    s2r = inv2
    c2r = (b2 * inv2 + be2 - m2 * inv2).astype(f32)

    w1bd = np.zeros((2, 128, 32), dtype=bf16)
    for q in range(2):
        for gh in range(4):
            g = 4 * q + gh
            w1bd[q, 32 * gh: 32 * gh + 32, 8 * gh: 8 * gh + 8] = w1[g].T

    w2c = np.zeros((64, 80), dtype=bf16)
    s2c = np.zeros((80, 1), dtype=f32)
    c2c = np.zeros((80, 1), dtype=f32)
    for k in range(9):
        for g in range(8):
            r = 8 + 9 * g + k
            gc, co = r // 10, r % 10
            w2c[8 * gc: 8 * gc + 8, 8 * k + g] = w2[gc, co, :]
            s2c[8 * k + g, 0] = s2r[r]
            c2c[8 * k + g, 0] = c2r[r]
    for g in range(8):
        gc, co = g // 10, g % 10
        w2c[8 * gc: 8 * gc + 8, 72 + g] = w2[gc, co, :]
        s2c[72 + g, 0] = s2r[g]
        c2c[72 + g, 0] = c2r[g]

    wvh = np.zeros((4, 128, 128), dtype=bf16)   # [2h*2q]
    for h in range(2):
        for q in range(2):
            for gh in range(4):
                g = 4 * q + gh
                for i in range(16):
                    wvh[2 * h + q, 32 * gh: 32 * gh + 32, 16 * g + i] = \
                        wv[g, 16 * h + i, :]

    dsum = np.zeros((72, 8), dtype=bf16)
    bc72 = np.zeros((8, 72), dtype=bf16)
    selk = np.zeros((9, 72, 128), dtype=bf16)
    for k in range(9):
        for g in range(8):
            dsum[8 * k + g, g] = 1
            bc72[g, 8 * k + g] = 1
            selk[k, 8 * k + g, 16 * g: 16 * g + 16] = 1

    xp = np.zeros((B, C, H + 2, W + 2), dtype=f32)
    xp[:, :, 1:-1, 1:-1] = x

    shards = []
    for b in range(B):
        for qh in range(4):
            xs = xp[b, :, qh * HS: qh * HS + HI, :]
            xs = np.ascontiguousarray(xs.reshape(2, 128, NIN))
            hm = np.ones((8, 2 * WP), dtype=f32)
            if qh == 0:
                hm[:, :WP] = 0
            if qh == 3:
                hm[:, WP:] = 0
            shards.append({
                "x": xs, "w1bd": w1bd, "s1": s1[:, None].copy(),
                "c1": c1[:, None].copy(), "w2c": w2c, "s2c": s2c, "c2c": c2c,
                "wvh": wvh, "dsum": dsum, "bc72": bc72, "selk": selk,
                "hmask": hm,
            })
    return shards


def kernel(**inputs):
    if "nc" not in _NC_CACHE:
        _NC_CACHE["nc"] = _build_nc()
    nc = _NC_CACHE["nc"]

    shards = _host_prep(**inputs)
    res = run_bass_kernel_spmd(nc, shards, core_ids=list(range(8)))

    out = np.zeros((B, C, H, W), dtype=np.float32)
    for i, r in enumerate(res.results):
        b, qh = divmod(i, 4)
        o = r["out"].astype(np.float32)  # [2, 128, NOUT]
        for h in range(2):
            rr = o[h].reshape(128, HS, WP)[:, :, 1: 1 + W]
            rr = rr.reshape(8, 16, HS, W)
            for g in range(8):
                ch = 32 * g + 16 * h
                out[b, ch: ch + 16, qh * HS:(qh + 1) * HS, :] = rr[g]
    return out
